# revision 5
# baseline (speedup 1.0000x reference)
"""Trainium2 Bass kernel for nn_LoRATACMLP4 (B=16,K=8,F=512,INCH=OUTCH=512,R=8).

Data-parallel over batch across 8 NeuronCores (2 batches per core).

Math (per batch b, slot k, token t):
    y    = mean_k(x @ W_ave.T) + b_ave          (mean commutes with linear)
    xp   = x @ W_pass.T + b_pass
    h    = gelu([xp, y])
    z    = h @ v / INCH ; lora = z @ u.T / R
    out  = gelu(h @ W_out.T + b_out + lora + b)

Device-side layout strategy: compute in transposed space (inch on SBUF
partitions).  x is host-cast to bf16 and loaded pre-transposed via the DMA
xbar transpose; first-layer biases are per-partition ACT biases; the output
bias rides the rank-9 LoRA matmul as a ones-row (uext row 8 = b_out + b[b]);
1/(INCH*R) is folded into v and 1/K into W_ave on the host.
"""

import sys

sys.path.insert(0, "/opt/trn_rl_repo")

import numpy as np
import ml_dtypes

BF16 = ml_dtypes.bfloat16

B, K, F, INCH, OUTCH, R = 16, 8, 512, 512, 512, 8
HD = INCH // 2
N_CORES = 8
BPC = B // N_CORES  # batches per core

_CACHE = {}


def _build_bass(n_iters=1):
    import concourse.bass as bass
    import concourse.mybir as mybir
    from concourse import bacc, tile
    from contextlib import nullcontext

    fp32 = mybir.dt.float32
    bf16 = mybir.dt.bfloat16
    AF = mybir.ActivationFunctionType

    nc = bacc.Bacc(None, target_bir_lowering=False)

    x_d = nc.declare_dram_parameter("x", [BPC, INCH, K * F], bf16, isOutput=False)
    v_d = nc.declare_dram_parameter("v", [BPC, INCH, K * R], bf16, isOutput=False)
    ue_d = nc.declare_dram_parameter("uext", [BPC, K, R + 1, OUTCH], bf16, isOutput=False)
    wp_d = nc.declare_dram_parameter("wpassT", [INCH, HD], bf16, isOutput=False)
    wa_d = nc.declare_dram_parameter("waveT", [INCH, HD], bf16, isOutput=False)
    wo_d = nc.declare_dram_parameter("woutT", [INCH, OUTCH], bf16, isOutput=False)
    bp_d = nc.declare_dram_parameter("bpass", [HD, 1], fp32, isOutput=False)
    ba_d = nc.declare_dram_parameter("bave", [HD, 1], fp32, isOutput=False)
    ones_d = nc.declare_dram_parameter("ones", [1, F], bf16, isOutput=False)
    out_d = nc.declare_dram_parameter("out", [BPC, K, F, OUTCH], fp32, isOutput=True)

    with tile.TileContext(nc) as tc:
        with (
            tc.tile_pool(name="consts", bufs=1) as cpool,
            tc.tile_pool(name="xt", bufs=8) as xt_pool,
            tc.tile_pool(name="vt", bufs=8) as vt_pool,
            tc.tile_pool(name="tree", bufs=2) as tree_pool,
            tc.tile_pool(name="xsum", bufs=8) as xsum_pool,
            tc.tile_pool(name="hp", bufs=6) as hp_pool,
            tc.tile_pool(name="ha", bufs=4) as ha_pool,
            tc.tile_pool(name="zu", bufs=3) as zu_pool,
            tc.tile_pool(name="osb", bufs=3) as osb_pool,
            tc.tile_pool(name="ps_mm", bufs=2, space="PSUM") as ps_mm,
            tc.tile_pool(name="ps_z", bufs=2, space="PSUM") as ps_z,
            tc.tile_pool(name="ps_o", bufs=4, space="PSUM") as ps_o,
            tc.For_i(0, n_iters, 1) if n_iters > 1 else nullcontext(),
        ):
            # persistent weights / biases
            wout_sb, wp_sb, wa_sb = [], [], []
            for c in range(4):
                w = cpool.tile([128, OUTCH], bf16, name=f"wo{c}", tag=f"wo{c}")
                nc.sync.dma_start(out=w[:], in_=wo_d[c * 128 : (c + 1) * 128, :])
                wout_sb.append(w)
                w = cpool.tile([128, HD], bf16, name=f"wp{c}", tag=f"wp{c}")
                nc.sync.dma_start(out=w[:], in_=wp_d[c * 128 : (c + 1) * 128, :])
                wp_sb.append(w)
                w = cpool.tile([128, HD], bf16, name=f"wa{c}", tag=f"wa{c}")
                nc.sync.dma_start(out=w[:], in_=wa_d[c * 128 : (c + 1) * 128, :])
                wa_sb.append(w)
            bp_sb, ba_sb = [], []
            for m in range(2):
                t = cpool.tile([128, 1], fp32, name=f"bp{m}", tag=f"bp{m}")
                nc.sync.dma_start(out=t[:], in_=bp_d[m * 128 : (m + 1) * 128, :])
                bp_sb.append(t)
                t = cpool.tile([128, 1], fp32, name=f"ba{m}", tag=f"ba{m}")
                nc.sync.dma_start(out=t[:], in_=ba_d[m * 128 : (m + 1) * 128, :])
                ba_sb.append(t)

            for b in range(BPC):
                # x arrives host-transposed: xts[c] = x[b]^T chunk -> [128 inch, (k,t)]
                xts = []
                for c in range(4):
                    xt = xt_pool.tile([128, K * F], bf16, tag="xt", name=f"xt{b}_{c}")
                    nc.sync.dma_start(
                        out=xt[:], in_=x_d[b, c * 128 : (c + 1) * 128, :]
                    )
                    xts.append(xt)
                vts = []
                for c in range(4):
                    vt = vt_pool.tile([128, K * R], bf16, tag="vt", name=f"vt{b}_{c}")
                    nc.sync.dma_start(out=vt[:], in_=v_d[b, c * 128 : (c + 1) * 128, :])
                    vts.append(vt)
                # sum over k (1/K folded into waveT)
                xsums = []
                for c in range(4):
                    t1 = tree_pool.tile([128, 4 * F], bf16, tag="t1", bufs=2, name=f"t1_{b}{c}")
                    nc.vector.tensor_add(t1[:], xts[c][:, 0 : 4 * F], xts[c][:, 4 * F : 8 * F])
                    t2 = tree_pool.tile([128, 2 * F], bf16, tag="t2", bufs=2, name=f"t2_{b}{c}")
                    nc.vector.tensor_add(t2[:], t1[:, 0 : 2 * F], t1[:, 2 * F : 4 * F])
                    xs = xsum_pool.tile([128, F], bf16, tag="xs", name=f"xs{b}_{c}")
                    nc.vector.tensor_add(xs[:], t2[:, 0:F], t2[:, F : 2 * F])
                    xsums.append(xs)
                # yT = (W_ave/8) @ xsumT ; ha = gelu(yT + b_ave)
                has_ = []
                for m in range(2):
                    ps = ps_mm.tile([128, F], fp32, tag="mm", name=f"psy{b}_{m}")
                    for c in range(4):
                        nc.tensor.matmul(
                            ps[:],
                            wa_sb[c][:, m * 128 : (m + 1) * 128],
                            xsums[c][:],
                            start=(c == 0),
                            stop=(c == 3),
                        )
                    ha = ha_pool.tile([128, F], bf16, tag="ha", name=f"ha{b}_{m}")
                    nc.scalar.activation(ha[:], ps[:], AF.Gelu, bias=ba_sb[m][:])
                    has_.append(ha)

                for k in range(K):
                    # xpT = W_pass @ xT ; hp = gelu(xpT + b_pass)
                    hcat = []
                    for m in range(2):
                        ps = ps_mm.tile([128, F], fp32, tag="mm", name=f"psp{b}{k}{m}")
                        for c in range(4):
                            nc.tensor.matmul(
                                ps[:],
                                wp_sb[c][:, m * 128 : (m + 1) * 128],
                                xts[c][:, k * F : (k + 1) * F],
                                start=(c == 0),
                                stop=(c == 3),
                            )
                        hp = hp_pool.tile([128, F], bf16, tag="hp", name=f"hp{b}{k}{m}")
                        nc.scalar.activation(hp[:], ps[:], AF.Gelu, bias=bp_sb[m][:])
                        hcat.append(hp)
                    hcat = hcat + has_  # inch chunks: [hp0, hp1, ha0, ha1]

                    # zT' = v'^T @ h^T  (scale pre-folded into v)
                    zps = ps_z.tile([R, F], fp32, tag="z", name=f"z{b}{k}")
                    for c in range(4):
                        nc.tensor.matmul(
                            zps[:],
                            vts[c][:, k * R : (k + 1) * R],
                            hcat[c][:],
                            start=(c == 0),
                            stop=(c == 3),
                        )
                    zext = zu_pool.tile([R + 1, F], bf16, tag="zext", name=f"ze{b}{k}")
                    nc.vector.tensor_copy(zext[0:R, :], zps[:])
                    nc.sync.dma_start(out=zext[R : R + 1, :], in_=ones_d[:])
                    ue = zu_pool.tile([R + 1, OUTCH], bf16, tag="ue", name=f"ue{b}{k}")
                    nc.sync.dma_start(out=ue[:], in_=ue_d[b, k])

                    osb = osb_pool.tile([128, 4, OUTCH], fp32, tag="osb", name=f"o{b}{k}")
                    for m in range(4):
                        po = ps_o.tile([128, OUTCH], fp32, tag="po", name=f"po{b}{k}{m}")
                        for c in range(4):
                            nc.tensor.matmul(
                                po[:],
                                hcat[c][:, m * 128 : (m + 1) * 128],
                                wout_sb[c][:],
                                start=(c == 0),
                                stop=False,
                            )
                        nc.tensor.matmul(
                            po[:],
                            zext[:, m * 128 : (m + 1) * 128],
                            ue[:],
                            start=False,
                            stop=True,
                        )
                        nc.scalar.activation(osb[:, m, :], po[:], AF.Gelu)
                    nc.sync.dma_start(
                        out=out_d[b, k].rearrange("(m p) o -> p m o", p=128),
                        in_=osb[:],
                    )
    nc.compile()
    return nc


def _prep_inputs(x, u, v, b, W_pass, b_pass, W_ave, b_ave, W_out, b_out):
    x = np.asarray(x, dtype=np.float32)
    u = np.asarray(u, dtype=np.float32)
    v = np.asarray(v, dtype=np.float32)
    b = np.asarray(b, dtype=np.float32)

    xb = np.ascontiguousarray(
        x.reshape(B, K * F, INCH).astype(BF16).transpose(0, 2, 1)
    )
    vb = np.ascontiguousarray(
        (v * (1.0 / (INCH * R))).transpose(0, 2, 1, 3).reshape(B, INCH, K * R)
    ).astype(BF16)
    bias_vec = np.asarray(b_out, dtype=np.float32)[None, :] + b[:, 0, 0, :]  # [B, OUTCH]
    uext = np.concatenate(
        [
            u.transpose(0, 1, 3, 2),  # [B, K, R, OUTCH]
            np.broadcast_to(bias_vec[:, None, None, :], (B, K, 1, OUTCH)),
        ],
        axis=2,
    ).astype(BF16)
    wpassT = np.ascontiguousarray(np.asarray(W_pass, dtype=np.float32).T).astype(BF16)
    waveT = np.ascontiguousarray(np.asarray(W_ave, dtype=np.float32).T / K).astype(BF16)
    woutT = np.ascontiguousarray(np.asarray(W_out, dtype=np.float32).T).astype(BF16)
    bp = np.asarray(b_pass, dtype=np.float32).reshape(HD, 1)
    ba = np.asarray(b_ave, dtype=np.float32).reshape(HD, 1)

    in_maps = []
    for i in range(N_CORES):
        sl = slice(i * BPC, (i + 1) * BPC)
        in_maps.append(
            dict(
                x=np.ascontiguousarray(xb[sl]),
                v=np.ascontiguousarray(vb[sl]),
                uext=np.ascontiguousarray(uext[sl]),
                wpassT=wpassT,
                waveT=waveT,
                woutT=woutT,
                bpass=bp,
                bave=ba,
                ones=np.ones((1, F), dtype=BF16),
            )
        )
    return in_maps


def run(inputs, trace=False, n_iters=1, **spmd_kwargs):
    from concourse.bass_utils import run_bass_kernel_spmd

    key = "nc" if n_iters == 1 else f"nc{n_iters}"
    if key not in _CACHE:
        _CACHE[key] = _build_bass(n_iters)
    nc = _CACHE[key]
    in_maps = _prep_inputs(**inputs)
    res = run_bass_kernel_spmd(
        nc, in_maps, list(range(N_CORES)), trace=trace, **spmd_kwargs
    )
    out = np.concatenate(
        [np.asarray(res.results[i]["out"], dtype=np.float32) for i in range(N_CORES)],
        axis=0,
    ).reshape(B, K, F, OUTCH)
    return out, res


def kernel(**inputs):
    out, _ = run(inputs, trace=False)
    return out



# revision 6
# speedup vs baseline: 1.4606x; 1.4606x over previous
"""Trainium2 Bass kernel for nn_LoRATACMLP4 (B=16,K=8,F=512,INCH=OUTCH=512,R=8).

Data-parallel over batch across 8 NeuronCores (2 batches per core).

Math (per batch b, slot k, token t):
    y    = mean_k(x @ W_ave.T) + b_ave          (mean commutes with linear)
    xp   = x @ W_pass.T + b_pass
    h    = gelu([xp, y])
    z    = h @ v / INCH ; lora = z @ u.T / R
    out  = gelu(h @ W_out.T + b_out + lora + b)

Key structure (v2):
  * compute in transposed space (inch on SBUF partitions); x host-cast to
    bf16 and pre-transposed.
  * the gelu(y) half of h is shared across all K slots, so its contribution
    to the output linear  C_y = gelu(y) @ W_out[:, 256:].T + b_out + b  is
    computed ONCE per batch (PE) and added per (b,k) on the DVE while the
    PSUM only accumulates the per-k half (xp @ W_out[:, :256].T + lora).
  * bias rides the C_y psum as a K=1 ones-row matmul; output is written
    bf16 and upcast on the host; output DMA goes on the GPSIMD (SWDGE)
    queue so it does not serialize behind the x loads on the sync ring.
"""

import sys

sys.path.insert(0, "/opt/trn_rl_repo")

import numpy as np
import ml_dtypes

BF16 = ml_dtypes.bfloat16

B, K, F, INCH, OUTCH, R = 16, 8, 512, 512, 512, 8
HD = INCH // 2
N_CORES = 8
BPC = B // N_CORES  # batches per core

_CACHE = {}


def _build_bass(n_iters=1):
    import concourse.bass as bass
    import concourse.mybir as mybir
    from concourse import bacc, tile
    from contextlib import nullcontext

    fp32 = mybir.dt.float32
    bf16 = mybir.dt.bfloat16
    AF = mybir.ActivationFunctionType

    nc = bacc.Bacc(None, target_bir_lowering=False)

    x_d = nc.declare_dram_parameter("x", [BPC, INCH, K * F], bf16, isOutput=False)
    v_d = nc.declare_dram_parameter("v", [BPC, INCH, K * R], bf16, isOutput=False)
    ut_d = nc.declare_dram_parameter("ut", [BPC, R, K * OUTCH], bf16, isOutput=False)
    wp_d = nc.declare_dram_parameter("wpassT", [INCH, HD], bf16, isOutput=False)
    wa_d = nc.declare_dram_parameter("waveT", [INCH, HD], bf16, isOutput=False)
    wo_d = nc.declare_dram_parameter("woutT", [INCH, OUTCH], bf16, isOutput=False)
    bp_d = nc.declare_dram_parameter("bpass", [HD, 1], fp32, isOutput=False)
    ba_d = nc.declare_dram_parameter("bave", [HD, 1], fp32, isOutput=False)
    bo_d = nc.declare_dram_parameter("bout", [BPC, 1, OUTCH], bf16, isOutput=False)
    ones_d = nc.declare_dram_parameter("ones", [1, 128], bf16, isOutput=False)
    out_d = nc.declare_dram_parameter("out", [BPC, K, F, OUTCH], bf16, isOutput=True)

    with tile.TileContext(nc) as tc:
        with (
            tc.tile_pool(name="consts", bufs=1) as cpool,
            tc.tile_pool(name="xt", bufs=8) as xt_pool,
            tc.tile_pool(name="vt", bufs=8) as vt_pool,
            tc.tile_pool(name="ut", bufs=2) as ut_pool,
            tc.tile_pool(name="tree", bufs=2) as tree_pool,
            tc.tile_pool(name="xsum", bufs=8) as xsum_pool,
            tc.tile_pool(name="hp", bufs=6) as hp_pool,
            tc.tile_pool(name="ha", bufs=4) as ha_pool,
            tc.tile_pool(name="cy", bufs=2) as cy_pool,
            tc.tile_pool(name="zu", bufs=3) as zu_pool,
            tc.tile_pool(name="ot", bufs=3) as ot_pool,
            tc.tile_pool(name="osb", bufs=3) as osb_pool,
            tc.tile_pool(name="ps_mm", bufs=3, space="PSUM") as ps_mm,
            tc.tile_pool(name="ps_z", bufs=1, space="PSUM") as ps_z,
            tc.tile_pool(name="ps_o", bufs=4, space="PSUM") as ps_o,
            tc.For_i(0, n_iters, 1) if n_iters > 1 else nullcontext(),
        ):
            # persistent weights / biases
            wout_sb, wp_sb, wa_sb = [], [], []
            for c in range(4):
                w = cpool.tile([128, OUTCH], bf16, name=f"wo{c}", tag=f"wo{c}")
                nc.sync.dma_start(out=w[:], in_=wo_d[c * 128 : (c + 1) * 128, :])
                wout_sb.append(w)
                w = cpool.tile([128, HD], bf16, name=f"wp{c}", tag=f"wp{c}")
                nc.sync.dma_start(out=w[:], in_=wp_d[c * 128 : (c + 1) * 128, :])
                wp_sb.append(w)
                w = cpool.tile([128, HD], bf16, name=f"wa{c}", tag=f"wa{c}")
                nc.sync.dma_start(out=w[:], in_=wa_d[c * 128 : (c + 1) * 128, :])
                wa_sb.append(w)
            bp_sb, ba_sb = [], []
            for m in range(2):
                t = cpool.tile([128, 1], fp32, name=f"bp{m}", tag=f"bp{m}")
                nc.sync.dma_start(out=t[:], in_=bp_d[m * 128 : (m + 1) * 128, :])
                bp_sb.append(t)
                t = cpool.tile([128, 1], fp32, name=f"ba{m}", tag=f"ba{m}")
                nc.sync.dma_start(out=t[:], in_=ba_d[m * 128 : (m + 1) * 128, :])
                ba_sb.append(t)
            ones_sb = cpool.tile([1, 128], bf16, name="ones", tag="ones")
            nc.sync.dma_start(out=ones_sb[:], in_=ones_d[:])

            for b in range(BPC):
                # x arrives host-transposed: xts[c] = x[b]^T chunk -> [128 inch, (k,t)]
                xts = []
                for c in range(4):
                    xt = xt_pool.tile([128, K * F], bf16, tag="xt", name=f"xt{b}_{c}")
                    nc.sync.dma_start(
                        out=xt[:], in_=x_d[b, c * 128 : (c + 1) * 128, :]
                    )
                    xts.append(xt)
                vts = []
                for c in range(4):
                    vt = vt_pool.tile([128, K * R], bf16, tag="vt", name=f"vt{b}_{c}")
                    nc.sync.dma_start(out=vt[:], in_=v_d[b, c * 128 : (c + 1) * 128, :])
                    vts.append(vt)
                # u^T for all K slots + bias row, one DMA each
                uts = ut_pool.tile([R, K * OUTCH], bf16, tag="ut", name=f"ut{b}")
                nc.sync.dma_start(out=uts[:], in_=ut_d[b])
                bo_sb = ut_pool.tile([1, OUTCH], bf16, tag="bo", name=f"bo{b}")
                nc.sync.dma_start(out=bo_sb[:], in_=bo_d[b])

                # sum over k (1/K folded into waveT)
                xsums = []
                for c in range(4):
                    t1 = tree_pool.tile([128, 4 * F], bf16, tag="t1", bufs=2, name=f"t1_{b}{c}")
                    nc.vector.tensor_add(t1[:], xts[c][:, 0 : 4 * F], xts[c][:, 4 * F : 8 * F])
                    t2 = tree_pool.tile([128, 2 * F], bf16, tag="t2", bufs=2, name=f"t2_{b}{c}")
                    nc.vector.tensor_add(t2[:], t1[:, 0 : 2 * F], t1[:, 2 * F : 4 * F])
                    xs = xsum_pool.tile([128, F], bf16, tag="xs", name=f"xs{b}_{c}")
                    nc.vector.tensor_add(xs[:], t2[:, 0:F], t2[:, F : 2 * F])
                    xsums.append(xs)
                # yT = (W_ave/8) @ xsumT ; ha = gelu(yT + b_ave)
                has_ = []
                for m in range(2):
                    ps = ps_mm.tile([128, F], fp32, tag="mm", name=f"psy{b}_{m}")
                    for c in range(4):
                        nc.tensor.matmul(
                            ps[:],
                            wa_sb[c][:, m * 128 : (m + 1) * 128],
                            xsums[c][:],
                            start=(c == 0),
                            stop=(c == 3),
                        )
                    ha = ha_pool.tile([128, F], bf16, tag="ha", name=f"ha{b}_{m}")
                    nc.scalar.activation(ha[:], ps[:], AF.Gelu, bias=ba_sb[m][:])
                    has_.append(ha)

                # C_y[f, o] = gelu(y) @ WoutT[256:, :] + (b_out + b[b])  (shared over k)
                cy = cy_pool.tile([128, 4 * OUTCH], fp32, tag="cy", name=f"cy{b}")
                for m in range(4):
                    ps = ps_o.tile([128, OUTCH], fp32, tag="po", name=f"pcy{b}{m}")
                    nc.tensor.matmul(
                        ps[:],
                        ones_sb[:],
                        bo_sb[:],
                        start=True,
                        stop=False,
                    )
                    for c in range(2):
                        nc.tensor.matmul(
                            ps[:],
                            has_[c][:, m * 128 : (m + 1) * 128],
                            wout_sb[2 + c][:],
                            start=False,
                            stop=(c == 1),
                        )
                    nc.vector.tensor_copy(cy[:, m * OUTCH : (m + 1) * OUTCH], ps[:])

                for k in range(K):
                    # xpT = W_pass @ xT ; hp = gelu(xpT + b_pass)
                    hcat = []
                    for m in range(2):
                        ps = ps_mm.tile([128, F], fp32, tag="mm", name=f"psp{b}{k}{m}")
                        for c in range(4):
                            nc.tensor.matmul(
                                ps[:],
                                wp_sb[c][:, m * 128 : (m + 1) * 128],
                                xts[c][:, k * F : (k + 1) * F],
                                start=(c == 0),
                                stop=(c == 3),
                            )
                        hp = hp_pool.tile([128, F], bf16, tag="hp", name=f"hp{b}{k}{m}")
                        nc.scalar.activation(hp[:], ps[:], AF.Gelu, bias=bp_sb[m][:])
                        hcat.append(hp)
                    hcat = hcat + has_  # inch chunks: [hp0, hp1, ha0, ha1]

                    # zT = v'^T @ h^T  (scale pre-folded into v)
                    zps = ps_z.tile([R, F], fp32, tag="z", name=f"z{b}{k}")
                    for c in range(4):
                        nc.tensor.matmul(
                            zps[:],
                            vts[c][:, k * R : (k + 1) * R],
                            hcat[c][:],
                            start=(c == 0),
                            stop=(c == 3),
                        )
                    zext = zu_pool.tile([R, F], bf16, tag="zext", name=f"ze{b}{k}")
                    nc.vector.tensor_copy(zext[:], zps[:])

                    # po[f, o] accumulates xp-part + lora only; C_y added on DVE
                    otmp = ot_pool.tile([128, 4 * OUTCH], bf16, tag="ot", name=f"ot{b}{k}")
                    for m in range(4):
                        po = ps_o.tile([128, OUTCH], fp32, tag="po", name=f"po{b}{k}{m}")
                        for c in range(2):
                            nc.tensor.matmul(
                                po[:],
                                hcat[c][:, m * 128 : (m + 1) * 128],
                                wout_sb[c][:],
                                start=(c == 0),
                                stop=False,
                            )
                        nc.tensor.matmul(
                            po[:],
                            zext[:, m * 128 : (m + 1) * 128],
                            uts[:, k * OUTCH : (k + 1) * OUTCH],
                            start=False,
                            stop=True,
                        )
                        nc.vector.tensor_add(
                            otmp[:, m * OUTCH : (m + 1) * OUTCH],
                            po[:],
                            cy[:, m * OUTCH : (m + 1) * OUTCH],
                        )
                    osb = osb_pool.tile([128, 4, OUTCH], bf16, tag="osb", name=f"o{b}{k}")
                    nc.scalar.activation(
                        osb[:].rearrange("p m o -> p (m o)"), otmp[:], AF.Gelu
                    )
                    nc.gpsimd.dma_start(
                        out=out_d[b, k].rearrange("(m p) o -> p m o", p=128),
                        in_=osb[:],
                    )
    nc.compile()
    return nc


def _prep_inputs(x, u, v, b, W_pass, b_pass, W_ave, b_ave, W_out, b_out):
    x = np.asarray(x, dtype=np.float32)
    u = np.asarray(u, dtype=np.float32)
    v = np.asarray(v, dtype=np.float32)
    b = np.asarray(b, dtype=np.float32)

    xb = np.ascontiguousarray(
        x.reshape(B, K * F, INCH).astype(BF16).transpose(0, 2, 1)
    )
    vb = np.ascontiguousarray(
        (v * (1.0 / (INCH * R))).transpose(0, 2, 1, 3).reshape(B, INCH, K * R)
    ).astype(BF16)
    # u^T with k-major free dim: [B, R, K*OUTCH]
    ub = np.ascontiguousarray(u.transpose(0, 3, 1, 2).reshape(B, R, K * OUTCH)).astype(
        BF16
    )
    bias_vec = np.asarray(b_out, dtype=np.float32)[None, :] + b[:, 0, 0, :]  # [B, OUTCH]
    bias_vec = bias_vec[:, None, :].astype(BF16)  # [B, 1, OUTCH]
    wpassT = np.ascontiguousarray(np.asarray(W_pass, dtype=np.float32).T).astype(BF16)
    waveT = np.ascontiguousarray(np.asarray(W_ave, dtype=np.float32).T / K).astype(BF16)
    woutT = np.ascontiguousarray(np.asarray(W_out, dtype=np.float32).T).astype(BF16)
    bp = np.asarray(b_pass, dtype=np.float32).reshape(HD, 1)
    ba = np.asarray(b_ave, dtype=np.float32).reshape(HD, 1)

    in_maps = []
    for i in range(N_CORES):
        sl = slice(i * BPC, (i + 1) * BPC)
        in_maps.append(
            dict(
                x=np.ascontiguousarray(xb[sl]),
                v=np.ascontiguousarray(vb[sl]),
                ut=np.ascontiguousarray(ub[sl]),
                wpassT=wpassT,
                waveT=waveT,
                woutT=woutT,
                bpass=bp,
                bave=ba,
                bout=np.ascontiguousarray(bias_vec[sl]),
                ones=np.ones((1, 128), dtype=BF16),
            )
        )
    return in_maps


def run(inputs, trace=False, n_iters=1, **spmd_kwargs):
    from concourse.bass_utils import run_bass_kernel_spmd

    key = "nc" if n_iters == 1 else f"nc{n_iters}"
    if key not in _CACHE:
        _CACHE[key] = _build_bass(n_iters)
    nc = _CACHE[key]
    in_maps = _prep_inputs(**inputs)
    res = run_bass_kernel_spmd(
        nc, in_maps, list(range(N_CORES)), trace=trace, **spmd_kwargs
    )
    out = np.concatenate(
        [np.asarray(res.results[i]["out"], dtype=np.float32) for i in range(N_CORES)],
        axis=0,
    ).reshape(B, K, F, OUTCH)
    return out, res


def kernel(**inputs):
    out, _ = run(inputs, trace=False)
    return out


# revision 7
# speedup vs baseline: 1.4684x; 1.0053x over previous
"""Trainium2 Bass kernel for nn_LoRATACMLP4 (B=16,K=8,F=512,INCH=OUTCH=512,R=8).

Data-parallel over batch across 8 NeuronCores (2 batches per core).

Math (per batch b, slot k, token t):
    y    = mean_k(x @ W_ave.T) + b_ave          (mean commutes with linear)
    xp   = x @ W_pass.T + b_pass
    h    = gelu([xp, y])
    z    = h @ v / INCH ; lora = z @ u.T / R
    out  = gelu(h @ W_out.T + b_out + lora + b)

Key structure (v3):
  * compute in transposed space (inch on SBUF partitions); x host-cast to
    bf16 and pre-transposed.
  * the gelu(y) half of h is shared across all K slots, so its contribution
    to the output linear  C_y = gelu(y) @ W_out[:, 256:].T + b_out + b  is
    computed ONCE per batch (PE) and added per (b,k) on the DVE while the
    PSUM only accumulates the per-k half (xp @ W_out[:, :256].T + lora).
  * software-pipelined k loop: PE stream per slot j is
    [z_j | pass_{j+1} | out_j] so the hp-gelu (ACT) and zext copy (DVE)
    latencies hide behind other PE work instead of stalling it.
  * bias rides the C_y psum as a K=1 ones-row matmul; output is written
    bf16 and upcast on the host; output DMA goes on the GPSIMD (SWDGE)
    queue so it does not serialize behind the x loads on the sync ring.
"""

import sys

sys.path.insert(0, "/opt/trn_rl_repo")

import numpy as np
import ml_dtypes

BF16 = ml_dtypes.bfloat16

B, K, F, INCH, OUTCH, R = 16, 8, 512, 512, 512, 8
HD = INCH // 2
N_CORES = 8
BPC = B // N_CORES  # batches per core
NSLOT = BPC * K

_CACHE = {}


def _build_bass(n_iters=1):
    import concourse.bass as bass
    import concourse.mybir as mybir
    from concourse import bacc, tile
    from contextlib import nullcontext

    fp32 = mybir.dt.float32
    bf16 = mybir.dt.bfloat16
    AF = mybir.ActivationFunctionType

    nc = bacc.Bacc(None, target_bir_lowering=False)

    x_d = nc.declare_dram_parameter("x", [BPC, INCH, K * F], bf16, isOutput=False)
    v_d = nc.declare_dram_parameter("v", [BPC, 128, 4 * K * R], bf16, isOutput=False)
    ut_d = nc.declare_dram_parameter("ut", [BPC, R, K * OUTCH], bf16, isOutput=False)
    wp_d = nc.declare_dram_parameter("wpassT", [INCH, HD], bf16, isOutput=False)
    wa_d = nc.declare_dram_parameter("waveT", [INCH, HD], bf16, isOutput=False)
    wo_d = nc.declare_dram_parameter("woutT", [INCH, OUTCH], bf16, isOutput=False)
    bp_d = nc.declare_dram_parameter("bpass", [HD, 1], fp32, isOutput=False)
    ba_d = nc.declare_dram_parameter("bave", [HD, 1], fp32, isOutput=False)
    bo_d = nc.declare_dram_parameter("bout", [BPC, 1, OUTCH], bf16, isOutput=False)
    ones_d = nc.declare_dram_parameter("ones", [1, 128], bf16, isOutput=False)
    out_d = nc.declare_dram_parameter("out", [BPC, K, F, OUTCH], bf16, isOutput=True)

    with tile.TileContext(nc) as tc:
        with (
            tc.tile_pool(name="consts", bufs=1) as cpool,
            tc.tile_pool(name="xt", bufs=8) as xt_pool,
            tc.tile_pool(name="vt", bufs=2) as vt_pool,
            tc.tile_pool(name="ut", bufs=2) as ut_pool,
            tc.tile_pool(name="tree", bufs=2) as tree_pool,
            tc.tile_pool(name="xsum", bufs=8) as xsum_pool,
            tc.tile_pool(name="hp", bufs=6) as hp_pool,
            tc.tile_pool(name="ha", bufs=4) as ha_pool,
            tc.tile_pool(name="cy", bufs=2) as cy_pool,
            tc.tile_pool(name="zu", bufs=3) as zu_pool,
            tc.tile_pool(name="ot", bufs=3) as ot_pool,
            tc.tile_pool(name="osb", bufs=3) as osb_pool,
            tc.tile_pool(name="ps_mm", bufs=3, space="PSUM") as ps_mm,
            tc.tile_pool(name="ps_z", bufs=1, space="PSUM") as ps_z,
            tc.tile_pool(name="ps_o", bufs=4, space="PSUM") as ps_o,
            tc.For_i(0, n_iters, 1) if n_iters > 1 else nullcontext(),
        ):
            # ---- persistent weights / biases ----
            wout_sb, wp_sb, wa_sb = [], [], []
            for c in range(4):
                w = cpool.tile([128, OUTCH], bf16, name=f"wo{c}", tag=f"wo{c}")
                nc.sync.dma_start(out=w[:], in_=wo_d[c * 128 : (c + 1) * 128, :])
                wout_sb.append(w)
                w = cpool.tile([128, HD], bf16, name=f"wp{c}", tag=f"wp{c}")
                nc.sync.dma_start(out=w[:], in_=wp_d[c * 128 : (c + 1) * 128, :])
                wp_sb.append(w)
                w = cpool.tile([128, HD], bf16, name=f"wa{c}", tag=f"wa{c}")
                nc.sync.dma_start(out=w[:], in_=wa_d[c * 128 : (c + 1) * 128, :])
                wa_sb.append(w)
            bp_sb, ba_sb = [], []
            for m in range(2):
                t = cpool.tile([128, 1], fp32, name=f"bp{m}", tag=f"bp{m}")
                nc.sync.dma_start(out=t[:], in_=bp_d[m * 128 : (m + 1) * 128, :])
                bp_sb.append(t)
                t = cpool.tile([128, 1], fp32, name=f"ba{m}", tag=f"ba{m}")
                nc.sync.dma_start(out=t[:], in_=ba_d[m * 128 : (m + 1) * 128, :])
                ba_sb.append(t)
            ones_sb = cpool.tile([1, 128], bf16, name="ones", tag="ones")
            nc.sync.dma_start(out=ones_sb[:], in_=ones_d[:])

            # ---- per-batch state ----
            xts = [None] * BPC  # x^T chunks
            vts = [None] * BPC  # v (4 chunks packed in free dim)
            uts = [None] * BPC  # u^T all slots
            has_ = [None] * BPC  # gelu(y)^T chunks
            cys = [None] * BPC  # C_y [128, 4*OUTCH]

            def load_batch(b):
                t = []
                for c in range(4):
                    xt = xt_pool.tile([128, K * F], bf16, tag="xt", name=f"xt{b}_{c}")
                    nc.sync.dma_start(out=xt[:], in_=x_d[b, c * 128 : (c + 1) * 128, :])
                    t.append(xt)
                xts[b] = t
                vt = vt_pool.tile([128, 4 * K * R], bf16, tag="vt", name=f"vt{b}")
                nc.sync.dma_start(out=vt[:], in_=v_d[b])
                vts[b] = vt
                ut = ut_pool.tile([R, K * OUTCH], bf16, tag="ut", name=f"ut{b}")
                nc.sync.dma_start(out=ut[:], in_=ut_d[b])
                bo_sb = ut_pool.tile([1, OUTCH], bf16, tag="bo", name=f"bo{b}")
                nc.sync.dma_start(out=bo_sb[:], in_=bo_d[b])
                uts[b] = (ut, bo_sb)

            def batch_head(b):
                """xsum tree + y matmuls + gelu + C_y build for batch b."""
                xsums = []
                for c in range(4):
                    t1 = tree_pool.tile(
                        [128, 4 * F], bf16, tag="t1", bufs=2, name=f"t1_{b}{c}"
                    )
                    nc.vector.tensor_add(
                        t1[:], xts[b][c][:, 0 : 4 * F], xts[b][c][:, 4 * F : 8 * F]
                    )
                    t2 = tree_pool.tile(
                        [128, 2 * F], bf16, tag="t2", bufs=2, name=f"t2_{b}{c}"
                    )
                    nc.vector.tensor_add(t2[:], t1[:, 0 : 2 * F], t1[:, 2 * F : 4 * F])
                    xs = xsum_pool.tile([128, F], bf16, tag="xs", name=f"xs{b}_{c}")
                    nc.vector.tensor_add(xs[:], t2[:, 0:F], t2[:, F : 2 * F])
                    xsums.append(xs)
                hh = []
                for m in range(2):
                    ps = ps_mm.tile([128, F], fp32, tag="mm", name=f"psy{b}_{m}")
                    for c in range(4):
                        nc.tensor.matmul(
                            ps[:],
                            wa_sb[c][:, m * 128 : (m + 1) * 128],
                            xsums[c][:],
                            start=(c == 0),
                            stop=(c == 3),
                        )
                    ha = ha_pool.tile([128, F], bf16, tag="ha", name=f"ha{b}_{m}")
                    nc.scalar.activation(ha[:], ps[:], AF.Gelu, bias=ba_sb[m][:])
                    hh.append(ha)
                has_[b] = hh
                # C_y[f, o] = gelu(y) @ WoutT[256:, :] + (b_out + b[b])
                cy = cy_pool.tile([128, 4 * OUTCH], fp32, tag="cy", name=f"cy{b}")
                for m in range(4):
                    ps = ps_o.tile([128, OUTCH], fp32, tag="po", name=f"pcy{b}{m}")
                    nc.tensor.matmul(
                        ps[:], ones_sb[:], uts[b][1][:], start=True, stop=False
                    )
                    for c in range(2):
                        nc.tensor.matmul(
                            ps[:],
                            hh[c][:, m * 128 : (m + 1) * 128],
                            wout_sb[2 + c][:],
                            start=False,
                            stop=(c == 1),
                        )
                    nc.vector.tensor_copy(cy[:, m * OUTCH : (m + 1) * OUTCH], ps[:])
                cys[b] = cy

            def emit_pass(b, k):
                """pass matmuls + hp gelu for slot (b,k); returns hp chunks."""
                hcat = []
                for m in range(2):
                    ps = ps_mm.tile([128, F], fp32, tag="mm", name=f"psp{b}{k}{m}")
                    for c in range(4):
                        nc.tensor.matmul(
                            ps[:],
                            wp_sb[c][:, m * 128 : (m + 1) * 128],
                            xts[b][c][:, k * F : (k + 1) * F],
                            start=(c == 0),
                            stop=(c == 3),
                        )
                    hp = hp_pool.tile([128, F], bf16, tag="hp", name=f"hp{b}{k}{m}")
                    nc.scalar.activation(hp[:], ps[:], AF.Gelu, bias=bp_sb[m][:])
                    hcat.append(hp)
                return hcat

            def emit_z(b, k, hcat):
                """z matmuls (PE); returns the psum tile."""
                zps = ps_z.tile([R, F], fp32, tag="z", name=f"z{b}{k}")
                hfull = hcat + has_[b]
                for c in range(4):
                    nc.tensor.matmul(
                        zps[:],
                        vts[b][:, (c * K + k) * R : (c * K + k) * R + R],
                        hfull[c][:],
                        start=(c == 0),
                        stop=(c == 3),
                    )
                return zps

            def emit_zext(b, k, zps):
                zext = zu_pool.tile([R, F], bf16, tag="zext", name=f"ze{b}{k}")
                nc.vector.tensor_copy(zext[:], zps[:])
                return zext

            def emit_out(b, k, hcat, zext):
                """out psum accumulation (PE) then DVE add, gelu, DMA."""
                otmp = ot_pool.tile([128, 4 * OUTCH], bf16, tag="ot", name=f"ot{b}{k}")
                for m in range(4):
                    po = ps_o.tile([128, OUTCH], fp32, tag="po", name=f"po{b}{k}{m}")
                    for c in range(2):
                        nc.tensor.matmul(
                            po[:],
                            hcat[c][:, m * 128 : (m + 1) * 128],
                            wout_sb[c][:],
                            start=(c == 0),
                            stop=False,
                        )
                    nc.tensor.matmul(
                        po[:],
                        zext[:, m * 128 : (m + 1) * 128],
                        uts[b][0][:, k * OUTCH : (k + 1) * OUTCH],
                        start=False,
                        stop=True,
                    )
                    nc.vector.tensor_add(
                        otmp[:, m * OUTCH : (m + 1) * OUTCH],
                        po[:],
                        cys[b][:, m * OUTCH : (m + 1) * OUTCH],
                    )
                osb = osb_pool.tile([128, 4, OUTCH], bf16, tag="osb", name=f"o{b}{k}")
                nc.scalar.activation(
                    osb[:].rearrange("p m o -> p (m o)"), otmp[:], AF.Gelu
                )
                nc.gpsimd.dma_start(
                    out=out_d[b, k].rearrange("(m p) o -> p m o", p=128),
                    in_=osb[:],
                )

            # ---- software-pipelined slot loop ----
            slots = [(b, k) for b in range(BPC) for k in range(K)]
            load_batch(0)
            if BPC > 1:
                load_batch(1)  # second batch loads overlap batch-0 compute
            batch_head(0)
            hps = {}
            hps[0] = emit_pass(*slots[0])
            for j, (b, k) in enumerate(slots):
                zps = emit_z(b, k, hps[j])
                nb_nk = slots[j + 1] if j + 1 < len(slots) else None
                if nb_nk is not None:
                    if nb_nk[1] == 0:  # entering next batch: its head first
                        batch_head(nb_nk[0])
                    hps[j + 1] = emit_pass(*nb_nk)
                zext = emit_zext(b, k, zps)
                emit_out(b, k, hps[j], zext)
                del hps[j]
    nc.compile()
    return nc


def _prep_inputs(x, u, v, b, W_pass, b_pass, W_ave, b_ave, W_out, b_out):
    x = np.asarray(x, dtype=np.float32)
    u = np.asarray(u, dtype=np.float32)
    v = np.asarray(v, dtype=np.float32)
    b = np.asarray(b, dtype=np.float32)

    xb = np.ascontiguousarray(
        x.reshape(B, K * F, INCH).astype(BF16).transpose(0, 2, 1)
    )
    # v: [B,K,INCH,R] -> [B, 128, (c k r)] with inch = c*128 + i
    vb = (
        (v * (1.0 / (INCH * R)))
        .transpose(0, 2, 1, 3)  # [B, INCH, K, R]
        .reshape(B, 4, 128, K * R)
        .transpose(0, 2, 1, 3)  # [B, 128, 4, K*R]
        .reshape(B, 128, 4 * K * R)
    )
    vb = np.ascontiguousarray(vb).astype(BF16)
    # u^T with k-major free dim: [B, R, K*OUTCH]
    ub = np.ascontiguousarray(u.transpose(0, 3, 1, 2).reshape(B, R, K * OUTCH)).astype(
        BF16
    )
    bias_vec = np.asarray(b_out, dtype=np.float32)[None, :] + b[:, 0, 0, :]  # [B, OUTCH]
    bias_vec = bias_vec[:, None, :].astype(BF16)  # [B, 1, OUTCH]
    wpassT = np.ascontiguousarray(np.asarray(W_pass, dtype=np.float32).T).astype(BF16)
    waveT = np.ascontiguousarray(np.asarray(W_ave, dtype=np.float32).T / K).astype(BF16)
    woutT = np.ascontiguousarray(np.asarray(W_out, dtype=np.float32).T).astype(BF16)
    bp = np.asarray(b_pass, dtype=np.float32).reshape(HD, 1)
    ba = np.asarray(b_ave, dtype=np.float32).reshape(HD, 1)

    in_maps = []
    for i in range(N_CORES):
        sl = slice(i * BPC, (i + 1) * BPC)
        in_maps.append(
            dict(
                x=np.ascontiguousarray(xb[sl]),
                v=np.ascontiguousarray(vb[sl]),
                ut=np.ascontiguousarray(ub[sl]),
                wpassT=wpassT,
                waveT=waveT,
                woutT=woutT,
                bpass=bp,
                bave=ba,
                bout=np.ascontiguousarray(bias_vec[sl]),
                ones=np.ones((1, 128), dtype=BF16),
            )
        )
    return in_maps


def run(inputs, trace=False, n_iters=1, **spmd_kwargs):
    from concourse.bass_utils import run_bass_kernel_spmd

    key = "nc" if n_iters == 1 else f"nc{n_iters}"
    if key not in _CACHE:
        _CACHE[key] = _build_bass(n_iters)
    nc = _CACHE[key]
    in_maps = _prep_inputs(**inputs)
    res = run_bass_kernel_spmd(
        nc, in_maps, list(range(N_CORES)), trace=trace, **spmd_kwargs
    )
    out = np.concatenate(
        [np.asarray(res.results[i]["out"], dtype=np.float32) for i in range(N_CORES)],
        axis=0,
    ).reshape(B, K, F, OUTCH)
    return out, res


def kernel(**inputs):
    out, _ = run(inputs, trace=False)
    return out


# revision 11
# speedup vs baseline: 1.5702x; 1.0693x over previous
"""Trainium2 Bass kernel for nn_LoRATACMLP4 (B=16,K=8,F=512,INCH=OUTCH=512,R=8).

Data-parallel over batch across 8 NeuronCores (2 batches per core).

Math (per batch b, slot k, token t):
    y    = mean_k(x @ W_ave.T) + b_ave          (mean commutes with linear)
    xp   = x @ W_pass.T + b_pass
    h    = gelu([xp, y])
    z    = h @ v / INCH ; lora = z @ u.T / R
    out  = gelu(h @ W_out.T + b_out + lora + b)

Key structure (v3):
  * compute in transposed space (inch on SBUF partitions); x host-cast to
    bf16 and pre-transposed.
  * the gelu(y) half of h is shared across all K slots, so its contribution
    to the output linear  C_y = gelu(y) @ W_out[:, 256:].T + b_out + b  is
    computed ONCE per batch (PE) and added per (b,k) on the DVE while the
    PSUM only accumulates the per-k half (xp @ W_out[:, :256].T + lora).
  * software-pipelined k loop: PE stream per slot j is
    [z_j | pass_{j+1} | out_j] so the hp-gelu (ACT) and zext copy (DVE)
    latencies hide behind other PE work instead of stalling it.
  * bias rides the C_y psum as a K=1 ones-row matmul; output is written
    bf16 and upcast on the host; output DMA goes on the GPSIMD (SWDGE)
    queue so it does not serialize behind the x loads on the sync ring.
"""

import sys

sys.path.insert(0, "/opt/trn_rl_repo")

import numpy as np
import ml_dtypes

BF16 = ml_dtypes.bfloat16

B, K, F, INCH, OUTCH, R = 16, 8, 512, 512, 512, 8
HD = INCH // 2
N_CORES = 8
BPC = B // N_CORES  # batches per core
NSLOT = BPC * K

_CACHE = {}


def _build_bass(n_iters=1):
    import concourse.bass as bass
    import concourse.mybir as mybir
    from concourse import bacc, tile
    from contextlib import nullcontext

    fp32 = mybir.dt.float32
    bf16 = mybir.dt.bfloat16
    AF = mybir.ActivationFunctionType

    nc = bacc.Bacc(None, target_bir_lowering=False)

    x_d = nc.declare_dram_parameter("x", [BPC, INCH, K * F], bf16, isOutput=False)
    v_d = nc.declare_dram_parameter("v", [BPC, 128, 4 * K * R], bf16, isOutput=False)
    ut_d = nc.declare_dram_parameter("ut", [BPC, R, K * OUTCH], bf16, isOutput=False)
    wp_d = nc.declare_dram_parameter("wpassT", [INCH, HD], bf16, isOutput=False)
    wa_d = nc.declare_dram_parameter("waveT", [INCH, HD], bf16, isOutput=False)
    wo_d = nc.declare_dram_parameter("woutT", [INCH, OUTCH], bf16, isOutput=False)
    bp_d = nc.declare_dram_parameter("bpass", [HD, 1], fp32, isOutput=False)
    ba_d = nc.declare_dram_parameter("bave", [HD, 1], fp32, isOutput=False)
    bo_d = nc.declare_dram_parameter("bout", [BPC, 1, OUTCH], bf16, isOutput=False)
    ones_d = nc.declare_dram_parameter("ones", [1, 128], bf16, isOutput=False)
    out_d = nc.declare_dram_parameter("out", [BPC, K, F, OUTCH], bf16, isOutput=True)

    with tile.TileContext(nc) as tc:
        with (
            tc.tile_pool(name="consts", bufs=1) as cpool,
            tc.tile_pool(name="xt", bufs=8) as xt_pool,
            tc.tile_pool(name="vt", bufs=2) as vt_pool,
            tc.tile_pool(name="ut", bufs=2) as ut_pool,
            tc.tile_pool(name="tree", bufs=2) as tree_pool,
            tc.tile_pool(name="xsum", bufs=8) as xsum_pool,
            tc.tile_pool(name="hp", bufs=6) as hp_pool,
            tc.tile_pool(name="ha", bufs=4) as ha_pool,
            tc.tile_pool(name="cy", bufs=2) as cy_pool,
            tc.tile_pool(name="zu", bufs=3) as zu_pool,
            tc.tile_pool(name="ot", bufs=3) as ot_pool,
            tc.tile_pool(name="osb", bufs=3) as osb_pool,
            tc.tile_pool(name="ps_mm", bufs=3, space="PSUM") as ps_mm,
            tc.tile_pool(name="ps_z", bufs=1, space="PSUM") as ps_z,
            tc.tile_pool(name="ps_o", bufs=4, space="PSUM") as ps_o,
            tc.For_i(0, n_iters, 1) if n_iters > 1 else nullcontext(),
        ):
            # ---- persistent weights / biases (gpsimd queue: off the x-load
            # critical path on the sync ring) ----
            wout_sb, wp_sb, wa_sb = [], [], []
            for c in range(4):
                w = cpool.tile([128, HD], bf16, name=f"wp{c}", tag=f"wp{c}")
                nc.gpsimd.dma_start(out=w[:], in_=wp_d[c * 128 : (c + 1) * 128, :])
                wp_sb.append(w)
            for c in range(4):
                w = cpool.tile([128, HD], bf16, name=f"wa{c}", tag=f"wa{c}")
                nc.gpsimd.dma_start(out=w[:], in_=wa_d[c * 128 : (c + 1) * 128, :])
                wa_sb.append(w)
            for c in range(4):
                w = cpool.tile([128, OUTCH], bf16, name=f"wo{c}", tag=f"wo{c}")
                nc.gpsimd.dma_start(out=w[:], in_=wo_d[c * 128 : (c + 1) * 128, :])
                wout_sb.append(w)
            bp_sb, ba_sb = [], []
            for m in range(2):
                t = cpool.tile([128, 1], fp32, name=f"bp{m}", tag=f"bp{m}")
                nc.gpsimd.dma_start(out=t[:], in_=bp_d[m * 128 : (m + 1) * 128, :])
                bp_sb.append(t)
                t = cpool.tile([128, 1], fp32, name=f"ba{m}", tag=f"ba{m}")
                nc.gpsimd.dma_start(out=t[:], in_=ba_d[m * 128 : (m + 1) * 128, :])
                ba_sb.append(t)
            ones_sb = cpool.tile([1, 128], bf16, name="ones", tag="ones")
            nc.gpsimd.dma_start(out=ones_sb[:], in_=ones_d[:])

            # ---- per-batch state ----
            xts = [None] * BPC  # x^T chunks
            vts = [None] * BPC  # v (4 chunks packed in free dim)
            uts = [None] * BPC  # u^T all slots
            has_ = [None] * BPC  # gelu(y)^T chunks
            cys = [None] * BPC  # C_y [128, 4*OUTCH]

            def load_batch(b):
                # split the 4 x chunks between the sync and gpsimd DMA
                # queues so the batch lands in ~half the time
                t = []
                for c in range(4):
                    xt = xt_pool.tile([128, K * F], bf16, tag="xt", name=f"xt{b}_{c}")
                    eng = nc.sync if c % 2 == 0 else nc.gpsimd
                    eng.dma_start(out=xt[:], in_=x_d[b, c * 128 : (c + 1) * 128, :])
                    t.append(xt)
                xts[b] = t
                vt = vt_pool.tile([128, 4 * K * R], bf16, tag="vt", name=f"vt{b}")
                nc.sync.dma_start(out=vt[:], in_=v_d[b])
                vts[b] = vt
                ut = ut_pool.tile([R, K * OUTCH], bf16, tag="ut", name=f"ut{b}")
                nc.sync.dma_start(out=ut[:], in_=ut_d[b])
                bo_sb = ut_pool.tile([1, OUTCH], bf16, tag="bo", name=f"bo{b}")
                nc.sync.dma_start(out=bo_sb[:], in_=bo_d[b])
                uts[b] = (ut, bo_sb)

            def batch_head(b):
                """xsum tree + y matmuls + gelu + C_y build for batch b."""
                xsums = []
                for c in range(4):
                    t1 = tree_pool.tile(
                        [128, 4 * F], bf16, tag="t1", bufs=2, name=f"t1_{b}{c}"
                    )
                    nc.vector.tensor_add(
                        t1[:], xts[b][c][:, 0 : 4 * F], xts[b][c][:, 4 * F : 8 * F]
                    )
                    t2 = tree_pool.tile(
                        [128, 2 * F], bf16, tag="t2", bufs=2, name=f"t2_{b}{c}"
                    )
                    nc.vector.tensor_add(t2[:], t1[:, 0 : 2 * F], t1[:, 2 * F : 4 * F])
                    xs = xsum_pool.tile([128, F], bf16, tag="xs", name=f"xs{b}_{c}")
                    nc.vector.tensor_add(xs[:], t2[:, 0:F], t2[:, F : 2 * F])
                    xsums.append(xs)
                hh = []
                for m in range(2):
                    ps = ps_mm.tile([128, F], fp32, tag="mm", name=f"psy{b}_{m}")
                    for c in range(4):
                        nc.tensor.matmul(
                            ps[:],
                            wa_sb[c][:, m * 128 : (m + 1) * 128],
                            xsums[c][:],
                            start=(c == 0),
                            stop=(c == 3),
                        )
                    ha = ha_pool.tile([128, F], bf16, tag="ha", name=f"ha{b}_{m}")
                    nc.scalar.activation(ha[:], ps[:], AF.Gelu, bias=ba_sb[m][:])
                    hh.append(ha)
                has_[b] = hh
                # C_y[f, o] = gelu(y) @ WoutT[256:, :] + (b_out + b[b])
                cy = cy_pool.tile([128, 4 * OUTCH], fp32, tag="cy", name=f"cy{b}")
                for m in range(4):
                    ps = ps_o.tile([128, OUTCH], fp32, tag="po", name=f"pcy{b}{m}")
                    nc.tensor.matmul(
                        ps[:], ones_sb[:], uts[b][1][:], start=True, stop=False
                    )
                    for c in range(2):
                        nc.tensor.matmul(
                            ps[:],
                            hh[c][:, m * 128 : (m + 1) * 128],
                            wout_sb[2 + c][:],
                            start=False,
                            stop=(c == 1),
                        )
                    nc.vector.tensor_copy(cy[:, m * OUTCH : (m + 1) * OUTCH], ps[:])
                cys[b] = cy

            def emit_pass(b, k):
                """pass matmuls + hp gelu for slot (b,k); returns hp chunks."""
                hcat = []
                for m in range(2):
                    ps = ps_mm.tile([128, F], fp32, tag="mm", name=f"psp{b}{k}{m}")
                    for c in range(4):
                        nc.tensor.matmul(
                            ps[:],
                            wp_sb[c][:, m * 128 : (m + 1) * 128],
                            xts[b][c][:, k * F : (k + 1) * F],
                            start=(c == 0),
                            stop=(c == 3),
                        )
                    hp = hp_pool.tile([128, F], bf16, tag="hp", name=f"hp{b}{k}{m}")
                    nc.scalar.activation(hp[:], ps[:], AF.Gelu, bias=bp_sb[m][:])
                    hcat.append(hp)
                return hcat

            def emit_z(b, k, hcat):
                """z matmuls (PE); returns the psum tile."""
                zps = ps_z.tile([R, F], fp32, tag="z", name=f"z{b}{k}")
                hfull = hcat + has_[b]
                for c in range(4):
                    nc.tensor.matmul(
                        zps[:],
                        vts[b][:, (c * K + k) * R : (c * K + k) * R + R],
                        hfull[c][:],
                        start=(c == 0),
                        stop=(c == 3),
                    )
                return zps

            def emit_zext(b, k, zps):
                zext = zu_pool.tile([R, F], bf16, tag="zext", name=f"ze{b}{k}")
                nc.vector.tensor_copy(zext[:], zps[:])
                return zext

            def emit_out(b, k, hcat, zext, split_tail=False):
                """out psum accumulation (PE) then DVE add, gelu, DMA."""
                otmp = ot_pool.tile([128, 4 * OUTCH], bf16, tag="ot", name=f"ot{b}{k}")
                osb = osb_pool.tile([128, 4, OUTCH], bf16, tag="osb", name=f"o{b}{k}")
                for m in range(4):
                    po = ps_o.tile([128, OUTCH], fp32, tag="po", name=f"po{b}{k}{m}")
                    for c in range(2):
                        nc.tensor.matmul(
                            po[:],
                            hcat[c][:, m * 128 : (m + 1) * 128],
                            wout_sb[c][:],
                            start=(c == 0),
                            stop=False,
                        )
                    nc.tensor.matmul(
                        po[:],
                        zext[:, m * 128 : (m + 1) * 128],
                        uts[b][0][:, k * OUTCH : (k + 1) * OUTCH],
                        start=False,
                        stop=True,
                    )
                    nc.vector.tensor_add(
                        otmp[:, m * OUTCH : (m + 1) * OUTCH],
                        po[:],
                        cys[b][:, m * OUTCH : (m + 1) * OUTCH],
                    )
                    if split_tail:
                        # drain epilogue per m-chunk so the final DMA starts
                        # as early as possible (shrinks the pipeline tail)
                        nc.scalar.activation(
                            osb[:, m, :], otmp[:, m * OUTCH : (m + 1) * OUTCH], AF.Gelu
                        )
                        nc.gpsimd.dma_start(
                            out=out_d[b, k].rearrange("(m p) o -> p m o", p=128)[
                                :, m, :
                            ],
                            in_=osb[:, m, :],
                        )
                if not split_tail:
                    nc.scalar.activation(
                        osb[:].rearrange("p m o -> p (m o)"), otmp[:], AF.Gelu
                    )
                    nc.gpsimd.dma_start(
                        out=out_d[b, k].rearrange("(m p) o -> p m o", p=128),
                        in_=osb[:],
                    )

            # ---- software-pipelined slot loop ----
            slots = [(b, k) for b in range(BPC) for k in range(K)]
            load_batch(0)
            if BPC > 1:
                load_batch(1)  # second batch loads overlap batch-0 compute
            batch_head(0)
            hps = {}
            hps[0] = emit_pass(*slots[0])
            for j, (b, k) in enumerate(slots):
                zps = emit_z(b, k, hps[j])
                nb_nk = slots[j + 1] if j + 1 < len(slots) else None
                if nb_nk is not None:
                    if nb_nk[1] == 0:  # entering next batch: its head first
                        batch_head(nb_nk[0])
                    hps[j + 1] = emit_pass(*nb_nk)
                zext = emit_zext(b, k, zps)
                emit_out(b, k, hps[j], zext, split_tail=(j >= len(slots) - 2))
                del hps[j]
    nc.compile()
    return nc


def _prep_inputs(x, u, v, b, W_pass, b_pass, W_ave, b_ave, W_out, b_out):
    x = np.asarray(x, dtype=np.float32)
    u = np.asarray(u, dtype=np.float32)
    v = np.asarray(v, dtype=np.float32)
    b = np.asarray(b, dtype=np.float32)

    xb = np.ascontiguousarray(
        x.reshape(B, K * F, INCH).astype(BF16).transpose(0, 2, 1)
    )
    # v: [B,K,INCH,R] -> [B, 128, (c k r)] with inch = c*128 + i
    vb = (
        (v * (1.0 / (INCH * R)))
        .transpose(0, 2, 1, 3)  # [B, INCH, K, R]
        .reshape(B, 4, 128, K * R)
        .transpose(0, 2, 1, 3)  # [B, 128, 4, K*R]
        .reshape(B, 128, 4 * K * R)
    )
    vb = np.ascontiguousarray(vb).astype(BF16)
    # u^T with k-major free dim: [B, R, K*OUTCH]
    ub = np.ascontiguousarray(u.transpose(0, 3, 1, 2).reshape(B, R, K * OUTCH)).astype(
        BF16
    )
    bias_vec = np.asarray(b_out, dtype=np.float32)[None, :] + b[:, 0, 0, :]  # [B, OUTCH]
    bias_vec = bias_vec[:, None, :].astype(BF16)  # [B, 1, OUTCH]
    wpassT = np.ascontiguousarray(np.asarray(W_pass, dtype=np.float32).T).astype(BF16)
    waveT = np.ascontiguousarray(np.asarray(W_ave, dtype=np.float32).T / K).astype(BF16)
    woutT = np.ascontiguousarray(np.asarray(W_out, dtype=np.float32).T).astype(BF16)
    bp = np.asarray(b_pass, dtype=np.float32).reshape(HD, 1)
    ba = np.asarray(b_ave, dtype=np.float32).reshape(HD, 1)

    in_maps = []
    for i in range(N_CORES):
        sl = slice(i * BPC, (i + 1) * BPC)
        in_maps.append(
            dict(
                x=np.ascontiguousarray(xb[sl]),
                v=np.ascontiguousarray(vb[sl]),
                ut=np.ascontiguousarray(ub[sl]),
                wpassT=wpassT,
                waveT=waveT,
                woutT=woutT,
                bpass=bp,
                bave=ba,
                bout=np.ascontiguousarray(bias_vec[sl]),
                ones=np.ones((1, 128), dtype=BF16),
            )
        )
    return in_maps


def run(inputs, trace=False, n_iters=1, **spmd_kwargs):
    from concourse.bass_utils import run_bass_kernel_spmd

    key = "nc" if n_iters == 1 else f"nc{n_iters}"
    if key not in _CACHE:
        _CACHE[key] = _build_bass(n_iters)
    nc = _CACHE[key]
    in_maps = _prep_inputs(**inputs)
    res = run_bass_kernel_spmd(
        nc, in_maps, list(range(N_CORES)), trace=trace, **spmd_kwargs
    )
    out = np.concatenate(
        [np.asarray(res.results[i]["out"], dtype=np.float32) for i in range(N_CORES)],
        axis=0,
    ).reshape(B, K, F, OUTCH)
    return out, res


def kernel(**inputs):
    out, _ = run(inputs, trace=False)
    return out
